# revision 2
# baseline (speedup 1.0000x reference)
"""DRAM attention (sparse page-retrieval attention) on 8 TRN2 NeuronCores.

Sharding: tensor-parallel over the 8 kv heads (1 kv head / core). Each core:
  1. streams its head's dram_k [65536,128] from HBM, mean-pools pages via
     PE matmuls against a fixed ones-mask -> transposed page sums [d, 1024],
     then a matvec with qsum gives per-head partial page scores [1, 1024]
  2. a 4KB AllReduce sums partial scores across the 8 cores (scores sum
     over kv heads in the reference)
  3. top-64 pages on-device (8 rounds of max/max_index/match_replace)
  4. dma_gather of the selected pages' K/V (quad-of-token granularity)
  5. prefix attention (4096 gathered tokens) + suffix attention (local
     cache 4096 + 32 new tokens, causal corner mask), both with max/sum
     softmax stats, combined exactly (flash-style LSE combine)
All attention math is order-invariant over tokens, so the gathered-token
permutation and unsorted page order do not change the result.
"""

import sys

sys.path.insert(0, "/opt/trn_rl_repo")

import math

import numpy as np

N_CORES = 8
NTOK = 65536
D = 128
PAGE = 64
NPAGES = NTOK // PAGE  # 1024
TOPK = 64
SEL = TOPK * PAGE  # 4096 selected tokens
LSUF = 4096 + 32  # suffix tokens (local + new)
LSUF_PAD = 33 * 128  # 4224
NEG = -1.0e30

_CACHE = {}


def _build_program():
    from concourse import bacc, mybir, tile
    from concourse.masks import make_identity

    f32 = mybir.dt.float32
    nc = bacc.Bacc(
        "TRN2",
        target_bir_lowering=False,
        debug=False,
        enable_asserts=False,
        num_devices=N_CORES,
    )

    k_head = nc.dram_tensor("k_head", [NTOK, D], f32, kind="ExternalInput")
    v4 = nc.dram_tensor("v4", [NTOK // 4, 4 * D], f32, kind="ExternalInput")
    qT = nc.dram_tensor("qT", [D, 128], f32, kind="ExternalInput")
    ksufT = nc.dram_tensor("ksufT", [D, LSUF_PAD], f32, kind="ExternalInput")
    vsuf = nc.dram_tensor("vsuf", [LSUF_PAD, D], f32, kind="ExternalInput")
    mask32 = nc.dram_tensor("mask32", [128, 32], f32, kind="ExternalInput")
    ones2 = nc.dram_tensor("ones2", [128, 2], f32, kind="ExternalInput")
    pconst = nc.dram_tensor("pconst", [128, 1], f32, kind="ExternalInput")
    ones128 = nc.dram_tensor("ones128", [1, 128], f32, kind="ExternalInput")
    out_d = nc.dram_tensor("out", [128, D], f32, kind="ExternalOutput")

    with tile.TileContext(nc) as tc:
        with (
            tc.tile_pool(name="const", bufs=1) as p_const,
            tc.tile_pool(name="stream", bufs=2) as p_stream,
            tc.tile_pool(name="suf", bufs=1) as p_suf,
            tc.tile_pool(name="pref", bufs=1) as p_pref,
            tc.tile_pool(name="pt", bufs=3) as p_pt,
            tc.tile_pool(name="ps_pagesum", bufs=2, space="PSUM") as ps1,
            tc.tile_pool(name="ps_mid", bufs=1, space="PSUM") as ps2,
            tc.tile_pool(name="ps_logits", bufs=2, space="PSUM") as ps3,
            tc.tile_pool(name="ps_tr", bufs=2, space="PSUM") as ps4,
            tc.tile_pool(name="ps_pv", bufs=1, space="PSUM") as ps5,
            tc.tile_pool(name="dram", bufs=2, space="DRAM") as p_dram,
        ):
            # ---------------- constants ----------------
            ident = p_const.tile([128, 128], f32)
            make_identity(nc, ident[:])
            qT_sb = p_const.tile([D, 128], f32)
            nc.sync.dma_start(qT_sb[:], qT[:])
            ones2_sb = p_const.tile([128, 2], f32)
            nc.sync.dma_start(ones2_sb[:], ones2[:])
            mask32_sb = p_const.tile([128, 32], f32)
            nc.sync.dma_start(mask32_sb[:], mask32[:])
            pconst_sb = p_const.tile([128, 1], f32)
            nc.sync.dma_start(pconst_sb[:], pconst[:])
            ones128_sb = p_const.tile([1, 128], f32)
            nc.sync.dma_start(ones128_sb[:], ones128[:])

            # ---------------- phase 1: stream K, page sums, scores ----------------
            page_sumsT = p_const.tile([D, NPAGES], f32)
            k_r3 = k_head.ap().rearrange("(n p) d -> p n d", p=128)  # [128,512,128]
            CH = 16  # chunks
            TPC = 512 // CH  # tiles per chunk = 32
            ps_tile = None
            for c in range(CH):
                buf = p_stream.tile([128, TPC, D], f32, tag="stream")
                nc.sync.dma_start(buf[:], k_r3[:, c * TPC : (c + 1) * TPC, :])
                for n in range(TPC):
                    i = c * TPC + n
                    g, m = divmod(i, 64)
                    if m == 0:
                        ps_tile = ps1.tile([128, 128], f32)
                    nc.tensor.matmul(
                        ps_tile[:, 2 * m : 2 * m + 2],
                        lhsT=buf[:, n, :],
                        rhs=ones2_sb[:],
                        start=True,
                        stop=True,
                    )
                    if m == 63:
                        nc.scalar.copy(
                            page_sumsT[:, g * 128 : (g + 1) * 128], ps_tile[:]
                        )

            qsum = p_const.tile([D, 1], f32)
            nc.vector.reduce_sum(out=qsum[:], in_=qT_sb[:], axis=mybir.AxisListType.X)

            scores_sb = p_const.tile([1, NPAGES], f32)
            for h in range(2):
                sc_ps = ps2.tile([1, 512], f32, tag="mid")
                nc.tensor.matmul(
                    sc_ps[:],
                    lhsT=qsum[:],
                    rhs=page_sumsT[:, h * 512 : (h + 1) * 512],
                    start=True,
                    stop=True,
                )
                nc.scalar.copy(scores_sb[:, h * 512 : (h + 1) * 512], sc_ps[:])

            # ---------------- all-reduce partial scores ----------------
            cc_in = p_dram.tile([NPAGES], f32)
            cc_out = p_dram.tile([NPAGES], f32)
            nc.sync.dma_start(cc_in[:], scores_sb[:])
            nc.gpsimd.collective_compute(
                "AllReduce",
                mybir.AluOpType.add,
                replica_groups=[list(range(N_CORES))],
                ins=[cc_in.opt()],
                outs=[cc_out.opt()],
            )
            scores_g = p_const.tile([1, NPAGES], f32)
            nc.sync.dma_start(scores_g[:], cc_out[:])

            # ---------------- top-64 pages ----------------
            work = p_const.tile([1, NPAGES], f32)
            nc.vector.tensor_copy(work[:], scores_g[:])
            vals8 = p_const.tile([1, 8], f32)
            idx8 = p_const.tile([1, 8], mybir.dt.uint32)
            idxf = p_const.tile([1, TOPK], f32)
            for r in range(TOPK // 8):
                nc.vector.max(out=vals8[:], in_=work[:])
                nc.vector.max_index(out=idx8[:], in_max=vals8[:], in_values=work[:])
                nc.vector.tensor_copy(idxf[:, 8 * r : 8 * r + 8], idx8[:])
                nc.vector.match_replace(
                    out=work[:],
                    in_to_replace=vals8[:],
                    in_values=work[:],
                    imm_value=-3.0e38,
                )

            # ---------------- build gather indices (quad granularity) ----------------
            # quad row = page*16 + (p % 16); idxs[p, s] wraps i = s*16 + (p%16)
            bc_ps = ps2.tile([128, TOPK], f32, tag="mid")
            nc.tensor.matmul(
                bc_ps[:], lhsT=ones128_sb[:], rhs=idxf[:], start=True, stop=True
            )
            idx16f = p_const.tile([128, TOPK], f32)
            nc.vector.tensor_scalar(
                out=idx16f[:],
                in0=bc_ps[:],
                scalar1=16.0,
                scalar2=pconst_sb[:],
                op0=mybir.AluOpType.mult,
                op1=mybir.AluOpType.add,
            )
            idx16 = p_const.tile([128, TOPK], mybir.dt.int16)
            nc.vector.tensor_copy(idx16[:], idx16f[:])

            # ---------------- gather selected K/V pages ----------------
            KQ = p_pref.tile([128, 8, 512], f32)
            VQ = p_pref.tile([128, 8, 512], f32)
            k4 = k_head.ap().rearrange("(a b) d -> a (b d)", b=4)  # [16384, 512]
            nc.gpsimd.dma_gather(
                out_ap=KQ[:],
                in_ap=k4,
                idxs_ap=idx16[:],
                num_idxs=1024,
                num_idxs_reg=1024,
                elem_size=512,
                queue_num=0,
            )
            nc.gpsimd.dma_gather(
                out_ap=VQ[:],
                in_ap=v4[:],
                idxs_ap=idx16[:],
                num_idxs=1024,
                num_idxs_reg=1024,
                elem_size=512,
                queue_num=0,
            )

            # ---------------- suffix attention (independent of topk) ----------------
            ksufT_sb = p_suf.tile([D, LSUF_PAD], f32)
            nc.sync.dma_start(ksufT_sb[:], ksufT[:])
            vsuf_sb = p_suf.tile([128, 33, D], f32)
            nc.sync.dma_start(
                vsuf_sb[:], vsuf.ap().rearrange("(n p) d -> p n d", p=128)
            )
            logits_s = p_suf.tile([128, LSUF_PAD], f32)
            for c in range(9):
                w = 512 if c < 8 else 128
                lg_ps = ps3.tile([128, 512], f32, tag="lg")
                nc.tensor.matmul(
                    lg_ps[:, :w],
                    lhsT=qT_sb[:],
                    rhs=ksufT_sb[:, 512 * c : 512 * c + w],
                    start=True,
                    stop=True,
                )
                nc.scalar.copy(logits_s[:, 512 * c : 512 * c + w], lg_ps[:, :w])
            # causal corner mask on the 32 new tokens + kill the padding
            nc.vector.tensor_add(
                logits_s[:, 4096:4128], logits_s[:, 4096:4128], mask32_sb[:]
            )
            nc.vector.memset(logits_s[:, 4128:LSUF_PAD], NEG)
            m_s = p_const.tile([128, 1], f32)
            nc.vector.reduce_max(out=m_s[:], in_=logits_s[:], axis=mybir.AxisListType.X)
            negm_s = p_const.tile([128, 1], f32)
            nc.vector.tensor_scalar_mul(negm_s[:], m_s[:], -1.0)
            sum_s = p_const.tile([128, 1], f32)
            nc.scalar.activation(
                out=logits_s[:],
                in_=logits_s[:],
                func=mybir.ActivationFunctionType.Exp,
                bias=negm_s[:],
                scale=1.0,
                accum_out=sum_s[:],
            )
            out_s_ps = ps5.tile([128, D], f32, tag="pv")
            for t in range(33):
                pt_ps = ps4.tile([128, 128], f32, tag="tr")
                nc.tensor.transpose(
                    pt_ps[:], logits_s[:, 128 * t : 128 * (t + 1)], ident[:]
                )
                pt_sb = p_pt.tile([128, 128], f32, tag="pt")
                nc.vector.tensor_copy(pt_sb[:], pt_ps[:])
                nc.tensor.matmul(
                    out_s_ps[:],
                    lhsT=pt_sb[:],
                    rhs=vsuf_sb[:, t, :],
                    start=(t == 0),
                    stop=(t == 32),
                )
            out_s_sb = p_const.tile([128, D], f32)
            nc.vector.tensor_copy(out_s_sb[:], out_s_ps[:])

            # ---------------- prefix attention over gathered pages ----------------
            kselT = p_pref.tile([D, SEL], f32)
            for t in range(32):
                b, r = divmod(t, 4)
                tr_ps = ps4.tile([128, 128], f32, tag="tr")
                nc.tensor.transpose(
                    tr_ps[:], KQ[:, b, 128 * r : 128 * (r + 1)], ident[:]
                )
                nc.vector.tensor_copy(kselT[:, 128 * t : 128 * (t + 1)], tr_ps[:])
            logits_p = p_pref.tile([128, SEL], f32)
            for c in range(8):
                lg_ps = ps3.tile([128, 512], f32, tag="lg")
                nc.tensor.matmul(
                    lg_ps[:],
                    lhsT=qT_sb[:],
                    rhs=kselT[:, 512 * c : 512 * (c + 1)],
                    start=True,
                    stop=True,
                )
                nc.scalar.copy(logits_p[:, 512 * c : 512 * (c + 1)], lg_ps[:])
            m_p = p_const.tile([128, 1], f32)
            nc.vector.reduce_max(out=m_p[:], in_=logits_p[:], axis=mybir.AxisListType.X)
            negm_p = p_const.tile([128, 1], f32)
            nc.vector.tensor_scalar_mul(negm_p[:], m_p[:], -1.0)
            sum_p = p_const.tile([128, 1], f32)
            nc.scalar.activation(
                out=logits_p[:],
                in_=logits_p[:],
                func=mybir.ActivationFunctionType.Exp,
                bias=negm_p[:],
                scale=1.0,
                accum_out=sum_p[:],
            )
            out_p_ps = ps5.tile([128, D], f32, tag="pv")
            for t in range(32):
                b, r = divmod(t, 4)
                pt_ps = ps4.tile([128, 128], f32, tag="tr")
                nc.tensor.transpose(
                    pt_ps[:], logits_p[:, 128 * t : 128 * (t + 1)], ident[:]
                )
                pt_sb = p_pt.tile([128, 128], f32, tag="pt")
                nc.vector.tensor_copy(pt_sb[:], pt_ps[:])
                nc.tensor.matmul(
                    out_p_ps[:],
                    lhsT=pt_sb[:],
                    rhs=VQ[:, b, 128 * r : 128 * (r + 1)],
                    start=(t == 0),
                    stop=(t == 31),
                )
            out_p_sb = p_const.tile([128, D], f32)
            nc.vector.tensor_copy(out_p_sb[:], out_p_ps[:])

            # ---------------- LSE-weighted combine ----------------
            M = p_const.tile([128, 1], f32)
            nc.vector.tensor_max(M[:], m_p[:], m_s[:])
            dp = p_const.tile([128, 1], f32)
            nc.vector.tensor_sub(dp[:], m_p[:], M[:])
            ep = p_const.tile([128, 1], f32)
            nc.scalar.activation(
                out=ep[:], in_=dp[:], func=mybir.ActivationFunctionType.Exp
            )
            ds_ = p_const.tile([128, 1], f32)
            nc.vector.tensor_sub(ds_[:], m_s[:], M[:])
            es = p_const.tile([128, 1], f32)
            nc.scalar.activation(
                out=es[:], in_=ds_[:], func=mybir.ActivationFunctionType.Exp
            )
            a_t = p_const.tile([128, 1], f32)
            nc.vector.tensor_mul(a_t[:], sum_p[:], ep[:])
            b_t = p_const.tile([128, 1], f32)
            nc.vector.tensor_mul(b_t[:], sum_s[:], es[:])
            den = p_const.tile([128, 1], f32)
            nc.vector.tensor_add(den[:], a_t[:], b_t[:])
            rden = p_const.tile([128, 1], f32)
            nc.vector.reciprocal(rden[:], den[:])
            w1 = p_const.tile([128, 1], f32)
            nc.vector.tensor_mul(w1[:], ep[:], rden[:])
            w2 = p_const.tile([128, 1], f32)
            nc.vector.tensor_mul(w2[:], es[:], rden[:])
            final_sb = p_const.tile([128, D], f32)
            nc.vector.tensor_scalar_mul(final_sb[:], out_p_sb[:], w1[:])
            nc.vector.scalar_tensor_tensor(
                out=final_sb[:],
                in0=out_s_sb[:],
                scalar=w2[:],
                in1=final_sb[:],
                op0=mybir.AluOpType.mult,
                op1=mybir.AluOpType.add,
            )
            nc.sync.dma_start(out_d[:], final_sb[:])

    nc.compile()
    return nc


def get_program():
    if "nc" not in _CACHE:
        _CACHE["nc"] = _build_program()
    return _CACHE["nc"]


def shard_inputs(xq, xk, xv, dram_k, dram_v, local_k, local_v):
    """Build the per-core input maps (host-side sharding by kv head)."""
    scale = 1.0 / math.sqrt(D)
    xq = np.asarray(xq, np.float32)
    xk = np.asarray(xk, np.float32)
    xv = np.asarray(xv, np.float32)
    dram_k = np.asarray(dram_k, np.float32)
    dram_v = np.asarray(dram_v, np.float32)
    local_k = np.asarray(local_k, np.float32)
    local_v = np.asarray(local_v, np.float32)

    mask32 = np.where(
        np.arange(32)[None, :] <= (np.arange(128) % 32)[:, None], 0.0, NEG
    ).astype(np.float32)
    ones2 = np.zeros((128, 2), np.float32)
    ones2[:64, 0] = 1.0
    ones2[64:, 1] = 1.0
    pconst = (np.arange(128) % 16).astype(np.float32).reshape(128, 1)
    ones128 = np.ones((1, 128), np.float32)

    in_maps = []
    for h in range(N_CORES):
        kh = np.ascontiguousarray(dram_k[0, :, h, :])  # [65536, 128]
        vh = np.ascontiguousarray(dram_v[0, :, h, :]).reshape(NTOK // 4, 4 * D)
        # rows r = g*32 + s, pre-scaled by 1/sqrt(D)
        q_rows = (
            xq[0, :, 4 * h : 4 * h + 4, :].transpose(1, 0, 2).reshape(128, D) * scale
        )
        qT_h = np.ascontiguousarray(q_rows.T)
        ksuf = np.concatenate([local_k[0, :, h, :], xk[0, :, h, :]], 0)  # [4128,128]
        ksuf_pad = np.zeros((LSUF_PAD, D), np.float32)
        ksuf_pad[:LSUF] = ksuf
        vsuf_pad = np.zeros((LSUF_PAD, D), np.float32)
        vsuf_pad[:LSUF] = np.concatenate([local_v[0, :, h, :], xv[0, :, h, :]], 0)
        in_maps.append(
            dict(
                k_head=kh,
                v4=vh,
                qT=np.ascontiguousarray(qT_h),
                ksufT=np.ascontiguousarray(ksuf_pad.T),
                vsuf=vsuf_pad,
                mask32=mask32,
                ones2=ones2,
                pconst=pconst,
                ones128=ones128,
            )
        )
    return in_maps


def unshard_output(results):
    out = np.zeros((1, 32, 32, 128), np.float32)
    for h in range(N_CORES):
        r = np.asarray(results[h]["out"])  # [128 rows, 128]; row = g*32 + s
        out[0, :, 4 * h : 4 * h + 4, :] = r.reshape(4, 32, D).transpose(1, 0, 2)
    return out


TRACE = False
LAST_RESULT = {}


def kernel(xq, xk, xv, dram_k, dram_v, local_k, local_v, start_pos=None, **_ignored):
    from concourse.bass_utils import run_bass_kernel_spmd

    nc = get_program()
    in_maps = shard_inputs(xq, xk, xv, dram_k, dram_v, local_k, local_v)
    res = run_bass_kernel_spmd(
        nc, in_maps, list(range(N_CORES)), trace=TRACE
    )
    LAST_RESULT["exec_time_ns"] = res.exec_time_ns
    LAST_RESULT["profile_json"] = res.profile_json
    return unshard_output(res.results)


# revision 9
# speedup vs baseline: 1.0138x; 1.0138x over previous
"""DRAM attention (sparse page-retrieval attention) on 8 TRN2 NeuronCores.

Sharding: tensor-parallel over the 8 kv heads (1 kv head / core). Each core:
  1. streams its head's dram_k [65536,128] from HBM, mean-pools pages via
     PE matmuls against a fixed ones-mask -> transposed page sums [d, 1024],
     then a matvec with qsum gives per-head partial page scores [1, 1024]
  2. a 4KB AllReduce sums partial scores across the 8 cores (scores sum
     over kv heads in the reference)
  3. top-64 pages on-device (8 rounds of max/max_index/match_replace)
  4. dma_gather of the selected pages' K/V (quad-of-token granularity)
  5. prefix attention (4096 gathered tokens) + suffix attention (local
     cache 4096 + 32 new tokens, causal corner mask), both with max/sum
     softmax stats, combined exactly (flash-style LSE combine)
All attention math is order-invariant over tokens, so the gathered-token
permutation and unsorted page order do not change the result.
"""

import sys

sys.path.insert(0, "/opt/trn_rl_repo")

import math

import numpy as np

N_CORES = 8
NTOK = 65536
D = 128
PAGE = 64
NPAGES = NTOK // PAGE  # 1024
TOPK = 64
SEL = TOPK * PAGE  # 4096 selected tokens
LSUF = 4096 + 32  # suffix tokens (local + new)
LSUF_PAD = 33 * 128  # 4224
NEG = -1.0e30

_CACHE = {}


def _build_program():
    from concourse import bacc, mybir, tile
    from concourse.masks import make_identity

    f32 = mybir.dt.float32
    nc = bacc.Bacc(
        "TRN2",
        target_bir_lowering=False,
        debug=False,
        enable_asserts=False,
        num_devices=N_CORES,
    )

    k_head = nc.dram_tensor("k_head", [NTOK, D], f32, kind="ExternalInput")
    v4 = nc.dram_tensor("v4", [NTOK // 4, 4 * D], f32, kind="ExternalInput")
    qT = nc.dram_tensor("qT", [D, 128], f32, kind="ExternalInput")
    ksufT = nc.dram_tensor("ksufT", [D, LSUF_PAD], f32, kind="ExternalInput")
    vsuf = nc.dram_tensor("vsuf", [LSUF_PAD, D], f32, kind="ExternalInput")
    mask32 = nc.dram_tensor("mask32", [128, 32], f32, kind="ExternalInput")
    ones2 = nc.dram_tensor("ones2", [128, 2], f32, kind="ExternalInput")
    pconst = nc.dram_tensor("pconst", [128, 1], f32, kind="ExternalInput")
    ones128 = nc.dram_tensor("ones128", [1, 128], f32, kind="ExternalInput")
    out_d = nc.dram_tensor("out", [128, D], f32, kind="ExternalOutput")

    with tile.TileContext(nc) as tc:
        with (
            tc.tile_pool(name="const", bufs=1) as p_const,
            tc.tile_pool(name="stream", bufs=2) as p_stream,
            tc.tile_pool(name="suf", bufs=1) as p_suf,
            tc.tile_pool(name="pref", bufs=1) as p_pref,
            tc.tile_pool(name="pt", bufs=3) as p_pt,
            tc.tile_pool(name="ps_pagesum", bufs=2, space="PSUM") as ps1,
            tc.tile_pool(name="ps_mid", bufs=1, space="PSUM") as ps2,
            tc.tile_pool(name="ps_logits", bufs=2, space="PSUM") as ps3,
            tc.tile_pool(name="ps_tr", bufs=2, space="PSUM") as ps4,
            tc.tile_pool(name="ps_pv", bufs=1, space="PSUM") as ps5,
            tc.tile_pool(name="dram", bufs=2, space="DRAM") as p_dram,
        ):
            # ---------------- constants ----------------
            ident = p_const.tile([128, 128], f32)
            make_identity(nc, ident[:])
            qT_sb = p_const.tile([D, 128], f32)
            nc.sync.dma_start(qT_sb[:], qT[:])
            ones2_sb = p_const.tile([128, 2], f32)
            nc.sync.dma_start(ones2_sb[:], ones2[:])
            mask32_sb = p_const.tile([128, 32], f32)
            nc.sync.dma_start(mask32_sb[:], mask32[:])
            pconst_sb = p_const.tile([128, 1], f32)
            nc.sync.dma_start(pconst_sb[:], pconst[:])
            ones128_sb = p_const.tile([1, 128], f32)
            nc.sync.dma_start(ones128_sb[:], ones128[:])

            # ---------- phase 1: stream K, page sums, scores ----------
            page_sumsT = p_const.tile([D, NPAGES], f32)
            k_r3 = k_head.ap().rearrange("(n p) d -> p n d", p=128)  # [128,512,128]
            CH = 16  # chunks
            TPC = 512 // CH  # tiles per chunk = 32
            ps_tile = None
            for c in range(CH):
                buf = p_stream.tile([128, TPC, D], f32, tag="stream")
                nc.sync.dma_start(buf[:], k_r3[:, c * TPC : (c + 1) * TPC, :])
                for n in range(TPC):
                    i = c * TPC + n
                    g, m = divmod(i, 64)
                    if m == 0:
                        ps_tile = ps1.tile([128, 128], f32)
                    nc.tensor.matmul(
                        ps_tile[:, 2 * m : 2 * m + 2],
                        lhsT=buf[:, n, :],
                        rhs=ones2_sb[:],
                        start=True,
                        stop=True,
                    )
                    if m == 63:
                        nc.scalar.copy(
                            page_sumsT[:, g * 128 : (g + 1) * 128], ps_tile[:]
                        )

            qsum = p_const.tile([D, 1], f32)
            nc.vector.reduce_sum(out=qsum[:], in_=qT_sb[:], axis=mybir.AxisListType.X)

            scores_sb = p_const.tile([1, NPAGES], f32)
            for h in range(2):
                sc_ps = ps2.tile([1, 512], f32, tag="mid")
                nc.tensor.matmul(
                    sc_ps[:],
                    lhsT=qsum[:],
                    rhs=page_sumsT[:, h * 512 : (h + 1) * 512],
                    start=True,
                    stop=True,
                )
                nc.scalar.copy(scores_sb[:, h * 512 : (h + 1) * 512], sc_ps[:])

            # ---------------- all-reduce partial scores ----------------
            cc_in = p_dram.tile([NPAGES], f32)
            cc_out = p_dram.tile([NPAGES], f32)
            nc.sync.dma_start(cc_in[:], scores_sb[:])
            nc.gpsimd.collective_compute(
                "AllReduce",
                mybir.AluOpType.add,
                replica_groups=[list(range(N_CORES))],
                ins=[cc_in.opt()],
                outs=[cc_out.opt()],
            )
            scores_g = p_const.tile([1, NPAGES], f32)
            nc.sync.dma_start(scores_g[:], cc_out[:])

            # ---------------- top-64 pages ----------------
            work = p_const.tile([1, NPAGES], f32)
            nc.vector.tensor_copy(work[:], scores_g[:])
            vals8 = p_const.tile([1, 8], f32)
            idx8 = p_const.tile([1, 8], mybir.dt.uint32)
            idxf = p_const.tile([1, TOPK], f32)
            for r in range(TOPK // 8):
                nc.vector.max(out=vals8[:], in_=work[:])
                nc.vector.max_index(out=idx8[:], in_max=vals8[:], in_values=work[:])
                nc.vector.tensor_copy(idxf[:, 8 * r : 8 * r + 8], idx8[:])
                nc.vector.match_replace(
                    out=work[:],
                    in_to_replace=vals8[:],
                    in_values=work[:],
                    imm_value=-3.0e38,
                )

            # ---------------- build gather indices (quad granularity) --------
            # quad row = page*16 + (p % 16); idxs[p, s] wraps i = s*16 + (p%16)
            bc_ps = ps2.tile([128, TOPK], f32, tag="mid")
            nc.tensor.matmul(
                bc_ps[:], lhsT=ones128_sb[:], rhs=idxf[:], start=True, stop=True
            )
            idx16f = p_const.tile([128, TOPK], f32)
            nc.vector.tensor_scalar(
                out=idx16f[:],
                in0=bc_ps[:],
                scalar1=16.0,
                scalar2=pconst_sb[:],
                op0=mybir.AluOpType.mult,
                op1=mybir.AluOpType.add,
            )
            idx16 = p_const.tile([128, TOPK], mybir.dt.int16)
            nc.vector.tensor_copy(idx16[:], idx16f[:])

            # ---------------- gather selected K/V pages ----------------
            KQ = p_pref.tile([128, 8, 512], f32)
            VQ = p_pref.tile([128, 8, 512], f32)
            k4 = k_head.ap().rearrange("(a b) d -> a (b d)", b=4)  # [16384, 512]
            nc.gpsimd.dma_gather(
                out_ap=KQ[:],
                in_ap=k4,
                idxs_ap=idx16[:],
                num_idxs=1024,
                num_idxs_reg=1024,
                elem_size=512,
                queue_num=0,
            )
            nc.gpsimd.dma_gather(
                out_ap=VQ[:],
                in_ap=v4[:],
                idxs_ap=idx16[:],
                num_idxs=1024,
                num_idxs_reg=1024,
                elem_size=512,
                queue_num=0,
            )

            # ---------------- suffix attention ----------------
            ksufT_sb = p_suf.tile([D, LSUF_PAD], f32)
            nc.sync.dma_start(ksufT_sb[:], ksufT[:])
            vsuf_sb = p_suf.tile([128, 33, D], f32)
            nc.sync.dma_start(
                vsuf_sb[:], vsuf.ap().rearrange("(n p) d -> p n d", p=128)
            )
            logits_s = p_suf.tile([128, LSUF_PAD], f32)
            for c in range(9):
                w = 512 if c < 8 else 128
                lg_ps = ps3.tile([128, 512], f32, tag="lg")
                nc.tensor.matmul(
                    lg_ps[:, :w],
                    lhsT=qT_sb[:],
                    rhs=ksufT_sb[:, 512 * c : 512 * c + w],
                    start=True,
                    stop=True,
                )
                nc.scalar.copy(logits_s[:, 512 * c : 512 * c + w], lg_ps[:, :w])
            # causal corner mask on the 32 new tokens + kill the padding
            nc.vector.tensor_add(
                logits_s[:, 4096:4128], logits_s[:, 4096:4128], mask32_sb[:]
            )
            nc.vector.memset(logits_s[:, 4128:LSUF_PAD], NEG)
            m_s = p_const.tile([128, 1], f32)
            nc.vector.reduce_max(out=m_s[:], in_=logits_s[:], axis=mybir.AxisListType.X)
            negm_s = p_const.tile([128, 1], f32)
            nc.vector.tensor_scalar_mul(negm_s[:], m_s[:], -1.0)
            sum_s = p_const.tile([128, 1], f32)
            nc.scalar.activation(
                out=logits_s[:],
                in_=logits_s[:],
                func=mybir.ActivationFunctionType.Exp,
                bias=negm_s[:],
                scale=1.0,
                accum_out=sum_s[:],
            )
            out_s_ps = ps5.tile([128, D], f32, tag="pv")
            for t in range(33):
                pt_ps = ps4.tile([128, 128], f32, tag="tr")
                nc.tensor.transpose(
                    pt_ps[:], logits_s[:, 128 * t : 128 * (t + 1)], ident[:]
                )
                pt_sb = p_pt.tile([128, 128], f32, tag="pt")
                nc.vector.tensor_copy(pt_sb[:], pt_ps[:])
                nc.tensor.matmul(
                    out_s_ps[:],
                    lhsT=pt_sb[:],
                    rhs=vsuf_sb[:, t, :],
                    start=(t == 0),
                    stop=(t == 32),
                )
            out_s_sb = p_const.tile([128, D], f32)
            nc.vector.tensor_copy(out_s_sb[:], out_s_ps[:])

            # ---------------- prefix attention over gathered pages ----------
            kselT = p_pref.tile([D, SEL], f32)
            for t in range(32):
                b, r = divmod(t, 4)
                tr_ps = ps4.tile([128, 128], f32, tag="tr")
                nc.tensor.transpose(
                    tr_ps[:], KQ[:, b, 128 * r : 128 * (r + 1)], ident[:]
                )
                nc.vector.tensor_copy(kselT[:, 128 * t : 128 * (t + 1)], tr_ps[:])
            logits_p = p_pref.tile([128, SEL], f32)
            for c in range(8):
                lg_ps = ps3.tile([128, 512], f32, tag="lg")
                nc.tensor.matmul(
                    lg_ps[:],
                    lhsT=qT_sb[:],
                    rhs=kselT[:, 512 * c : 512 * (c + 1)],
                    start=True,
                    stop=True,
                )
                nc.scalar.copy(logits_p[:, 512 * c : 512 * (c + 1)], lg_ps[:])
            m_p = p_const.tile([128, 1], f32)
            nc.vector.reduce_max(out=m_p[:], in_=logits_p[:], axis=mybir.AxisListType.X)
            negm_p = p_const.tile([128, 1], f32)
            nc.vector.tensor_scalar_mul(negm_p[:], m_p[:], -1.0)
            sum_p = p_const.tile([128, 1], f32)
            nc.scalar.activation(
                out=logits_p[:],
                in_=logits_p[:],
                func=mybir.ActivationFunctionType.Exp,
                bias=negm_p[:],
                scale=1.0,
                accum_out=sum_p[:],
            )
            out_p_ps = ps5.tile([128, D], f32, tag="pv")
            for t in range(32):
                b, r = divmod(t, 4)
                pt_ps = ps4.tile([128, 128], f32, tag="tr")
                nc.tensor.transpose(
                    pt_ps[:], logits_p[:, 128 * t : 128 * (t + 1)], ident[:]
                )
                pt_sb = p_pt.tile([128, 128], f32, tag="pt")
                nc.vector.tensor_copy(pt_sb[:], pt_ps[:])
                nc.tensor.matmul(
                    out_p_ps[:],
                    lhsT=pt_sb[:],
                    rhs=VQ[:, b, 128 * r : 128 * (r + 1)],
                    start=(t == 0),
                    stop=(t == 31),
                )
            out_p_sb = p_const.tile([128, D], f32)
            nc.vector.tensor_copy(out_p_sb[:], out_p_ps[:])

            # ---------------- LSE-weighted combine ----------------
            M = p_const.tile([128, 1], f32)
            nc.vector.tensor_max(M[:], m_p[:], m_s[:])
            dp = p_const.tile([128, 1], f32)
            nc.vector.tensor_sub(dp[:], m_p[:], M[:])
            ep = p_const.tile([128, 1], f32)
            nc.scalar.activation(
                out=ep[:], in_=dp[:], func=mybir.ActivationFunctionType.Exp
            )
            ds_ = p_const.tile([128, 1], f32)
            nc.vector.tensor_sub(ds_[:], m_s[:], M[:])
            es = p_const.tile([128, 1], f32)
            nc.scalar.activation(
                out=es[:], in_=ds_[:], func=mybir.ActivationFunctionType.Exp
            )
            a_t = p_const.tile([128, 1], f32)
            nc.vector.tensor_mul(a_t[:], sum_p[:], ep[:])
            b_t = p_const.tile([128, 1], f32)
            nc.vector.tensor_mul(b_t[:], sum_s[:], es[:])
            den = p_const.tile([128, 1], f32)
            nc.vector.tensor_add(den[:], a_t[:], b_t[:])
            rden = p_const.tile([128, 1], f32)
            nc.vector.reciprocal(rden[:], den[:])
            w1 = p_const.tile([128, 1], f32)
            nc.vector.tensor_mul(w1[:], ep[:], rden[:])
            w2 = p_const.tile([128, 1], f32)
            nc.vector.tensor_mul(w2[:], es[:], rden[:])
            final_sb = p_const.tile([128, D], f32)
            nc.vector.tensor_scalar_mul(final_sb[:], out_p_sb[:], w1[:])
            nc.vector.scalar_tensor_tensor(
                out=final_sb[:],
                in0=out_s_sb[:],
                scalar=w2[:],
                in1=final_sb[:],
                op0=mybir.AluOpType.mult,
                op1=mybir.AluOpType.add,
            )
            nc.sync.dma_start(out_d[:], final_sb[:])

    nc.compile()
    return nc


def get_program():
    if "nc" not in _CACHE:
        _CACHE["nc"] = _build_program()
    return _CACHE["nc"]


def shard_inputs(xq, xk, xv, dram_k, dram_v, local_k, local_v):
    """Build the per-core input maps (host-side sharding by kv head)."""
    scale = 1.0 / math.sqrt(D)
    xq = np.asarray(xq, np.float32)
    xk = np.asarray(xk, np.float32)
    xv = np.asarray(xv, np.float32)
    dram_k = np.asarray(dram_k, np.float32)
    dram_v = np.asarray(dram_v, np.float32)
    local_k = np.asarray(local_k, np.float32)
    local_v = np.asarray(local_v, np.float32)

    mask32 = np.where(
        np.arange(32)[None, :] <= (np.arange(128) % 32)[:, None], 0.0, NEG
    ).astype(np.float32)
    ones2 = np.zeros((128, 2), np.float32)
    ones2[:64, 0] = 1.0
    ones2[64:, 1] = 1.0
    pconst = (np.arange(128) % 16).astype(np.float32).reshape(128, 1)
    ones128 = np.ones((1, 128), np.float32)

    in_maps = []
    for h in range(N_CORES):
        kh = np.ascontiguousarray(dram_k[0, :, h, :])  # [65536, 128]
        vh = np.ascontiguousarray(dram_v[0, :, h, :]).reshape(NTOK // 4, 4 * D)
        # rows r = g*32 + s, pre-scaled by 1/sqrt(D)
        q_rows = (
            xq[0, :, 4 * h : 4 * h + 4, :].transpose(1, 0, 2).reshape(128, D) * scale
        )
        qT_h = np.ascontiguousarray(q_rows.T)
        ksuf = np.concatenate([local_k[0, :, h, :], xk[0, :, h, :]], 0)  # [4128,128]
        ksuf_pad = np.zeros((LSUF_PAD, D), np.float32)
        ksuf_pad[:LSUF] = ksuf
        vsuf_pad = np.zeros((LSUF_PAD, D), np.float32)
        vsuf_pad[:LSUF] = np.concatenate([local_v[0, :, h, :], xv[0, :, h, :]], 0)
        in_maps.append(
            dict(
                k_head=kh,
                v4=vh,
                qT=qT_h,
                ksufT=np.ascontiguousarray(ksuf_pad.T),
                vsuf=vsuf_pad,
                mask32=mask32,
                ones2=ones2,
                pconst=pconst,
                ones128=ones128,
            )
        )
    return in_maps


def unshard_output(results):
    out = np.zeros((1, 32, 32, 128), np.float32)
    for h in range(N_CORES):
        r = np.asarray(results[h]["out"])  # [128 rows, 128]; row = g*32 + s
        out[0, :, 4 * h : 4 * h + 4, :] = r.reshape(4, 32, D).transpose(1, 0, 2)
    return out


TRACE = False
LAST_RESULT = {}


def kernel(xq, xk, xv, dram_k, dram_v, local_k, local_v, start_pos=None, **_ignored):
    from concourse.bass_utils import run_bass_kernel_spmd

    nc = get_program()
    in_maps = shard_inputs(xq, xk, xv, dram_k, dram_v, local_k, local_v)
    res = run_bass_kernel_spmd(
        nc, in_maps, list(range(N_CORES)), trace=TRACE
    )
    LAST_RESULT["exec_time_ns"] = res.exec_time_ns
    LAST_RESULT["profile_json"] = res.profile_json
    return unshard_output(res.results)


# revision 10
# speedup vs baseline: 1.2443x; 1.2273x over previous
"""DRAM attention (sparse page-retrieval attention) on 8 TRN2 NeuronCores.

Sharding: tensor-parallel over the 8 kv heads (1 kv head / core). Each core:
  1. streams its head's dram_k [65536,128] from HBM, mean-pools pages via
     PE matmuls against a fixed ones-mask -> transposed page sums [d, 1024],
     then a matvec with qsum gives per-head partial page scores [1, 1024]
  2. a 4KB AllReduce sums partial scores across the 8 cores (scores sum
     over kv heads in the reference)
  3. top-64 pages on-device (8 rounds of max/max_index/match_replace)
  4. dma_gather of the selected pages' K/V (quad-of-token granularity)
  5. prefix attention (4096 gathered tokens) + suffix attention (local
     cache 4096 + 32 new tokens, causal corner mask), both with max/sum
     softmax stats, combined exactly (flash-style LSE combine)
All attention math is order-invariant over tokens, so the gathered-token
permutation and unsorted page order do not change the result.
"""

import sys

sys.path.insert(0, "/opt/trn_rl_repo")

import math

import numpy as np

N_CORES = 8
NTOK = 65536
D = 128
PAGE = 64
NPAGES = NTOK // PAGE  # 1024
TOPK = 64
SEL = TOPK * PAGE  # 4096 selected tokens
LSUF = 4096 + 32  # suffix tokens (local + new)
LSUF_PAD = 33 * 128  # 4224
NEG = -1.0e30

_CACHE = {}


def _build_program():
    from concourse import bacc, mybir, tile
    from concourse.masks import make_identity

    f32 = mybir.dt.float32
    f32r = mybir.dt.float32r
    nc = bacc.Bacc(
        "TRN2",
        target_bir_lowering=False,
        debug=False,
        enable_asserts=False,
        num_devices=N_CORES,
    )

    k_head = nc.dram_tensor("k_head", [NTOK, D], f32, kind="ExternalInput")
    v4 = nc.dram_tensor("v4", [NTOK // 4, 4 * D], f32, kind="ExternalInput")
    qT = nc.dram_tensor("qT", [D, 128], f32r, kind="ExternalInput")
    ksufT = nc.dram_tensor("ksufT", [D, LSUF_PAD], f32r, kind="ExternalInput")
    vsuf = nc.dram_tensor("vsuf", [LSUF_PAD, D], f32r, kind="ExternalInput")
    mask32 = nc.dram_tensor("mask32", [128, 32], f32, kind="ExternalInput")
    ones2 = nc.dram_tensor("ones2", [128, 2], f32r, kind="ExternalInput")
    pconst = nc.dram_tensor("pconst", [128, 1], f32, kind="ExternalInput")
    ones128 = nc.dram_tensor("ones128", [1, 128], f32, kind="ExternalInput")
    out_d = nc.dram_tensor("out", [128, D], f32, kind="ExternalOutput")

    with tile.TileContext(nc) as tc:
        with (
            tc.tile_pool(name="const", bufs=1) as p_const,
            tc.tile_pool(name="stream", bufs=2) as p_stream,
            tc.tile_pool(name="suf", bufs=1) as p_suf,
            tc.tile_pool(name="pref", bufs=1) as p_pref,
            tc.tile_pool(name="pt", bufs=3) as p_pt,
            tc.tile_pool(name="ps_pagesum", bufs=2, space="PSUM") as ps1,
            tc.tile_pool(name="ps_mid", bufs=1, space="PSUM") as ps2,
            tc.tile_pool(name="ps_logits", bufs=2, space="PSUM") as ps3,
            tc.tile_pool(name="ps_tr", bufs=2, space="PSUM") as ps4,
            tc.tile_pool(name="ps_pv", bufs=1, space="PSUM") as ps5,
            tc.tile_pool(name="dram", bufs=2, space="DRAM") as p_dram,
        ):
            # ---------------- constants ----------------
            ident = p_const.tile([128, 128], f32)
            make_identity(nc, ident[:])
            qT_sb = p_const.tile([D, 128], f32r)
            nc.sync.dma_start(qT_sb[:], qT[:])
            ones2_sb = p_const.tile([128, 2], f32r)
            nc.sync.dma_start(ones2_sb[:], ones2[:])
            mask32_sb = p_const.tile([128, 32], f32)
            nc.sync.dma_start(mask32_sb[:], mask32[:])
            pconst_sb = p_const.tile([128, 1], f32)
            nc.sync.dma_start(pconst_sb[:], pconst[:])
            ones128_sb = p_const.tile([1, 128], f32)
            nc.sync.dma_start(ones128_sb[:], ones128[:])

            # ---------- phase 1: stream K, page sums, scores ----------
            page_sumsT = p_const.tile([D, NPAGES], f32)
            k_r3 = k_head.ap().rearrange("(n p) d -> p n d", p=128)  # [128,512,128]
            CH = 16  # chunks
            TPC = 512 // CH  # tiles per chunk = 32
            ps_tile = None
            for c in range(CH):
                buf = p_stream.tile([128, TPC, D], f32r, tag="stream")
                nc.sync.dma_start(buf[:], k_r3[:, c * TPC : (c + 1) * TPC, :].bitcast(f32r))
                for n in range(TPC):
                    i = c * TPC + n
                    g, m = divmod(i, 64)
                    if m == 0:
                        ps_tile = ps1.tile([128, 128], f32)
                    nc.tensor.matmul(
                        ps_tile[:, 2 * m : 2 * m + 2],
                        lhsT=buf[:, n, :],
                        rhs=ones2_sb[:],
                        start=True,
                        stop=True,
                    )
                    if m == 63:
                        nc.scalar.copy(
                            page_sumsT[:, g * 128 : (g + 1) * 128], ps_tile[:]
                        )

            qsum = p_const.tile([D, 1], f32)
            nc.vector.reduce_sum(out=qsum[:], in_=qT_sb[:].bitcast(f32), axis=mybir.AxisListType.X)

            scores_sb = p_const.tile([1, NPAGES], f32)
            for h in range(2):
                sc_ps = ps2.tile([1, 512], f32, tag="mid")
                nc.tensor.matmul(
                    sc_ps[:],
                    lhsT=qsum[:],
                    rhs=page_sumsT[:, h * 512 : (h + 1) * 512],
                    start=True,
                    stop=True,
                )
                nc.scalar.copy(scores_sb[:, h * 512 : (h + 1) * 512], sc_ps[:])

            # ---------------- all-reduce partial scores ----------------
            cc_in = p_dram.tile([NPAGES], f32)
            cc_out = p_dram.tile([NPAGES], f32)
            nc.sync.dma_start(cc_in[:], scores_sb[:])
            nc.gpsimd.collective_compute(
                "AllReduce",
                mybir.AluOpType.add,
                replica_groups=[list(range(N_CORES))],
                ins=[cc_in.opt()],
                outs=[cc_out.opt()],
            )
            scores_g = p_const.tile([1, NPAGES], f32)
            nc.sync.dma_start(scores_g[:], cc_out[:])

            # ---------------- top-64 pages ----------------
            work = p_const.tile([1, NPAGES], f32)
            nc.vector.tensor_copy(work[:], scores_g[:])
            vals8 = p_const.tile([1, 8], f32)
            idx8 = p_const.tile([1, 8], mybir.dt.uint32)
            idxf = p_const.tile([1, TOPK], f32)
            for r in range(TOPK // 8):
                nc.vector.max(out=vals8[:], in_=work[:])
                nc.vector.max_index(out=idx8[:], in_max=vals8[:], in_values=work[:])
                nc.vector.tensor_copy(idxf[:, 8 * r : 8 * r + 8], idx8[:])
                nc.vector.match_replace(
                    out=work[:],
                    in_to_replace=vals8[:],
                    in_values=work[:],
                    imm_value=-3.0e38,
                )

            # ---------------- build gather indices (quad granularity) --------
            # quad row = page*16 + (p % 16); idxs[p, s] wraps i = s*16 + (p%16)
            bc_ps = ps2.tile([128, TOPK], f32, tag="mid")
            nc.tensor.matmul(
                bc_ps[:], lhsT=ones128_sb[:], rhs=idxf[:], start=True, stop=True
            )
            idx16f = p_const.tile([128, TOPK], f32)
            nc.vector.tensor_scalar(
                out=idx16f[:],
                in0=bc_ps[:],
                scalar1=16.0,
                scalar2=pconst_sb[:],
                op0=mybir.AluOpType.mult,
                op1=mybir.AluOpType.add,
            )
            idx16 = p_const.tile([128, TOPK], mybir.dt.int16)
            nc.vector.tensor_copy(idx16[:], idx16f[:])

            # ---------------- gather selected K/V pages ----------------
            KQ = p_pref.tile([128, 8, 512], f32r)
            VQ = p_pref.tile([128, 8, 512], f32r)
            k4 = k_head.ap().rearrange("(a b) d -> a (b d)", b=4).bitcast(f32r)
            nc.gpsimd.dma_gather(
                out_ap=KQ[:],
                in_ap=k4,
                idxs_ap=idx16[:],
                num_idxs=1024,
                num_idxs_reg=1024,
                elem_size=512,
                queue_num=0,
            )
            nc.gpsimd.dma_gather(
                out_ap=VQ[:],
                in_ap=v4[:].bitcast(f32r),
                idxs_ap=idx16[:],
                num_idxs=1024,
                num_idxs_reg=1024,
                elem_size=512,
                queue_num=0,
            )

            # ---------------- suffix attention ----------------
            ksufT_sb = p_suf.tile([D, LSUF_PAD], f32r)
            nc.sync.dma_start(ksufT_sb[:], ksufT[:])
            vsuf_sb = p_suf.tile([128, 33, D], f32r)
            nc.sync.dma_start(
                vsuf_sb[:], vsuf.ap().rearrange("(n p) d -> p n d", p=128)
            )
            logits_s = p_suf.tile([128, LSUF_PAD], f32)
            for c in range(9):
                w = 512 if c < 8 else 128
                lg_ps = ps3.tile([128, 512], f32, tag="lg")
                nc.tensor.matmul(
                    lg_ps[:, :w],
                    lhsT=qT_sb[:],
                    rhs=ksufT_sb[:, 512 * c : 512 * c + w],
                    start=True,
                    stop=True,
                )
                nc.scalar.copy(logits_s[:, 512 * c : 512 * c + w], lg_ps[:, :w])
            # causal corner mask on the 32 new tokens + kill the padding
            nc.vector.tensor_add(
                logits_s[:, 4096:4128], logits_s[:, 4096:4128], mask32_sb[:]
            )
            nc.vector.memset(logits_s[:, 4128:LSUF_PAD], NEG)
            m_s = p_const.tile([128, 1], f32)
            nc.vector.reduce_max(out=m_s[:], in_=logits_s[:], axis=mybir.AxisListType.X)
            negm_s = p_const.tile([128, 1], f32)
            nc.vector.tensor_scalar_mul(negm_s[:], m_s[:], -1.0)
            sum_s = p_const.tile([128, 1], f32)
            nc.scalar.activation(
                out=logits_s[:],
                in_=logits_s[:],
                func=mybir.ActivationFunctionType.Exp,
                bias=negm_s[:],
                scale=1.0,
                accum_out=sum_s[:],
            )
            out_s_ps = ps5.tile([128, D], f32, tag="pv")
            for t in range(33):
                pt_ps = ps4.tile([128, 128], f32, tag="tr")
                nc.tensor.transpose(
                    pt_ps[:], logits_s[:, 128 * t : 128 * (t + 1)], ident[:]
                )
                pt_sb = p_pt.tile([128, 128], f32r, tag="pt")
                nc.vector.tensor_copy(pt_sb[:], pt_ps[:])
                nc.tensor.matmul(
                    out_s_ps[:],
                    lhsT=pt_sb[:],
                    rhs=vsuf_sb[:, t, :],
                    start=(t == 0),
                    stop=(t == 32),
                )
            out_s_sb = p_const.tile([128, D], f32)
            nc.vector.tensor_copy(out_s_sb[:], out_s_ps[:])

            # ---------------- prefix attention over gathered pages ----------
            kselT = p_pref.tile([D, SEL], f32r)
            for t in range(32):
                b, r = divmod(t, 4)
                tr_ps = ps4.tile([128, 128], f32, tag="tr")
                nc.tensor.transpose(
                    tr_ps[:], KQ[:, b, 128 * r : 128 * (r + 1)].bitcast(f32), ident[:]
                )
                nc.vector.tensor_copy(kselT[:, 128 * t : 128 * (t + 1)], tr_ps[:])
            logits_p = p_pref.tile([128, SEL], f32)
            for c in range(8):
                lg_ps = ps3.tile([128, 512], f32, tag="lg")
                nc.tensor.matmul(
                    lg_ps[:],
                    lhsT=qT_sb[:],
                    rhs=kselT[:, 512 * c : 512 * (c + 1)],
                    start=True,
                    stop=True,
                )
                nc.scalar.copy(logits_p[:, 512 * c : 512 * (c + 1)], lg_ps[:])
            m_p = p_const.tile([128, 1], f32)
            nc.vector.reduce_max(out=m_p[:], in_=logits_p[:], axis=mybir.AxisListType.X)
            negm_p = p_const.tile([128, 1], f32)
            nc.vector.tensor_scalar_mul(negm_p[:], m_p[:], -1.0)
            sum_p = p_const.tile([128, 1], f32)
            nc.scalar.activation(
                out=logits_p[:],
                in_=logits_p[:],
                func=mybir.ActivationFunctionType.Exp,
                bias=negm_p[:],
                scale=1.0,
                accum_out=sum_p[:],
            )
            out_p_ps = ps5.tile([128, D], f32, tag="pv")
            for t in range(32):
                b, r = divmod(t, 4)
                pt_ps = ps4.tile([128, 128], f32, tag="tr")
                nc.tensor.transpose(
                    pt_ps[:], logits_p[:, 128 * t : 128 * (t + 1)], ident[:]
                )
                pt_sb = p_pt.tile([128, 128], f32r, tag="pt")
                nc.vector.tensor_copy(pt_sb[:], pt_ps[:])
                nc.tensor.matmul(
                    out_p_ps[:],
                    lhsT=pt_sb[:],
                    rhs=VQ[:, b, 128 * r : 128 * (r + 1)],
                    start=(t == 0),
                    stop=(t == 31),
                )
            out_p_sb = p_const.tile([128, D], f32)
            nc.vector.tensor_copy(out_p_sb[:], out_p_ps[:])

            # ---------------- LSE-weighted combine ----------------
            M = p_const.tile([128, 1], f32)
            nc.vector.tensor_max(M[:], m_p[:], m_s[:])
            dp = p_const.tile([128, 1], f32)
            nc.vector.tensor_sub(dp[:], m_p[:], M[:])
            ep = p_const.tile([128, 1], f32)
            nc.scalar.activation(
                out=ep[:], in_=dp[:], func=mybir.ActivationFunctionType.Exp
            )
            ds_ = p_const.tile([128, 1], f32)
            nc.vector.tensor_sub(ds_[:], m_s[:], M[:])
            es = p_const.tile([128, 1], f32)
            nc.scalar.activation(
                out=es[:], in_=ds_[:], func=mybir.ActivationFunctionType.Exp
            )
            a_t = p_const.tile([128, 1], f32)
            nc.vector.tensor_mul(a_t[:], sum_p[:], ep[:])
            b_t = p_const.tile([128, 1], f32)
            nc.vector.tensor_mul(b_t[:], sum_s[:], es[:])
            den = p_const.tile([128, 1], f32)
            nc.vector.tensor_add(den[:], a_t[:], b_t[:])
            rden = p_const.tile([128, 1], f32)
            nc.vector.reciprocal(rden[:], den[:])
            w1 = p_const.tile([128, 1], f32)
            nc.vector.tensor_mul(w1[:], ep[:], rden[:])
            w2 = p_const.tile([128, 1], f32)
            nc.vector.tensor_mul(w2[:], es[:], rden[:])
            final_sb = p_const.tile([128, D], f32)
            nc.vector.tensor_scalar_mul(final_sb[:], out_p_sb[:], w1[:])
            nc.vector.scalar_tensor_tensor(
                out=final_sb[:],
                in0=out_s_sb[:],
                scalar=w2[:],
                in1=final_sb[:],
                op0=mybir.AluOpType.mult,
                op1=mybir.AluOpType.add,
            )
            nc.sync.dma_start(out_d[:], final_sb[:])

    nc.compile()
    return nc


def get_program():
    if "nc" not in _CACHE:
        _CACHE["nc"] = _build_program()
    return _CACHE["nc"]


def shard_inputs(xq, xk, xv, dram_k, dram_v, local_k, local_v):
    """Build the per-core input maps (host-side sharding by kv head)."""
    scale = 1.0 / math.sqrt(D)
    xq = np.asarray(xq, np.float32)
    xk = np.asarray(xk, np.float32)
    xv = np.asarray(xv, np.float32)
    dram_k = np.asarray(dram_k, np.float32)
    dram_v = np.asarray(dram_v, np.float32)
    local_k = np.asarray(local_k, np.float32)
    local_v = np.asarray(local_v, np.float32)

    mask32 = np.where(
        np.arange(32)[None, :] <= (np.arange(128) % 32)[:, None], 0.0, NEG
    ).astype(np.float32)
    ones2 = np.zeros((128, 2), np.float32)
    ones2[:64, 0] = 1.0
    ones2[64:, 1] = 1.0
    pconst = (np.arange(128) % 16).astype(np.float32).reshape(128, 1)
    ones128 = np.ones((1, 128), np.float32)

    in_maps = []
    for h in range(N_CORES):
        kh = np.ascontiguousarray(dram_k[0, :, h, :])  # [65536, 128]
        vh = np.ascontiguousarray(dram_v[0, :, h, :]).reshape(NTOK // 4, 4 * D)
        # rows r = g*32 + s, pre-scaled by 1/sqrt(D)
        q_rows = (
            xq[0, :, 4 * h : 4 * h + 4, :].transpose(1, 0, 2).reshape(128, D) * scale
        )
        qT_h = np.ascontiguousarray(q_rows.T)
        ksuf = np.concatenate([local_k[0, :, h, :], xk[0, :, h, :]], 0)  # [4128,128]
        ksuf_pad = np.zeros((LSUF_PAD, D), np.float32)
        ksuf_pad[:LSUF] = ksuf
        vsuf_pad = np.zeros((LSUF_PAD, D), np.float32)
        vsuf_pad[:LSUF] = np.concatenate([local_v[0, :, h, :], xv[0, :, h, :]], 0)
        in_maps.append(
            dict(
                k_head=kh,
                v4=vh,
                qT=qT_h,
                ksufT=np.ascontiguousarray(ksuf_pad.T),
                vsuf=vsuf_pad,
                mask32=mask32,
                ones2=ones2,
                pconst=pconst,
                ones128=ones128,
            )
        )
    return in_maps


def unshard_output(results):
    out = np.zeros((1, 32, 32, 128), np.float32)
    for h in range(N_CORES):
        r = np.asarray(results[h]["out"])  # [128 rows, 128]; row = g*32 + s
        out[0, :, 4 * h : 4 * h + 4, :] = r.reshape(4, 32, D).transpose(1, 0, 2)
    return out


TRACE = False
LAST_RESULT = {}


def kernel(xq, xk, xv, dram_k, dram_v, local_k, local_v, start_pos=None, **_ignored):
    from concourse.bass_utils import run_bass_kernel_spmd

    nc = get_program()
    in_maps = shard_inputs(xq, xk, xv, dram_k, dram_v, local_k, local_v)
    res = run_bass_kernel_spmd(
        nc, in_maps, list(range(N_CORES)), trace=TRACE
    )
    LAST_RESULT["exec_time_ns"] = res.exec_time_ns
    LAST_RESULT["profile_json"] = res.profile_json
    return unshard_output(res.results)
